# revision 23
# baseline (speedup 1.0000x reference)
"""Trainium2 Bass kernel for CuteInferMLP: E = gelu(X @ W0^T + b0) @ W1^T + b1.

Full shapes: x (2, 2048, 2048) f32, W0 (8192, 2048), b0 (8192,),
W1 (2048, 8192), b1 (2048,). Output (2, 2048, 2048) f16.

Sharding: 8-way data-parallel over the 4096 tokens (512 tokens/core).
Each core holds the full weights and computes its token slice end to
end; the host just concatenates the 8 slices.

The kernel is PE-bound (fp16 matmul streams 1 moving row/cycle;
fp8e4m3 DoubleRow covers 2 k-slabs per instruction at the same row
rate = 2x MACs). Full fp8 fails the 2e-2 accuracy gate (~5% rel), so
a quarter of GEMM0's MACs and 1/32 of GEMM1's run in fp8 — sized so
total rel err ~1.85% < 2e-2. Quantization-error energy is additive
and position-independent, so GEMM0's fp8 budget is packed into its
FIRST 16 of 64 output blocks (fully fp8) instead of spread across all
blocks: the first ~27us of PE work then depends only on ~1.25MB of
fp8 inputs, taking the 2MB fp16 x stream off the startup critical
path entirely. Weight planes are pre-scaled by powers of two (exact
in fp16 and e4m3) so fp8 and fp16 partial sums share one PSUM scale,
dequantized for free via the activation's scale parameter.

Startup schedule (measured-optimal): the PE clock ramps over its
first ~5us of activity, so early work runs at ~60% speed. Keep early
PE work minimal and let DMA waits overlap the slow-clock phase;
front-running extra work into the ramp window consistently lost time.

Device layout per core (weights stationary, contraction on partitions):
  GEMM0: D^T[n,m] += W0T[h,n]-stationary @ X^T[h,m]   (h: 16 k-slabs)
         blocks n<16: 8 fp8 DR instrs; blocks n>=16: 16 fp16 matmuls
  act:   D^T = gelu(PSUM/1024 + b0) -> fp16 (ScalarE); first 2 blocks
         also cast to fp8 (VectorE) for GEMM1's DR instructions.
  GEMM1: E^T[hh,m] += W1T[n,hh]-stationary @ D^T[n,m] (n: 64 k-slabs,
         first pair fp8 DR, rest fp16)
  act:   E^T = PSUM/2048 + b1 -> fp16, DMA out. Last block split in
         three along tokens so its activation+DMA overlaps matmuls.
"""

import numpy as np
import ml_dtypes

from concourse import bacc, tile, mybir
from concourse.bass_utils import run_bass_kernel_spmd

P = 128
N_CORES = 8
B, L, H, N = 2, 2048, 2048, 8192
M = B * L                 # 4096 tokens
M_CORE = M // N_CORES     # 512 tokens per core
KB0 = H // P              # 16  k-slabs in GEMM0 (contraction over H)
NB = N // P               # 64  n-blocks (GEMM0 output partitions)
KB1 = N // P              # 64  k-slabs in GEMM1 (contraction over N)
HB = H // P               # 16  output blocks (GEMM1 output partitions)

NBQ = 16                  # GEMM0 blocks computed fully in fp8 DR
NBF = NB - NBQ            # GEMM0 blocks computed fully in fp16
KP0 = KB0 // 2            # DR k-pair count per fp8 block (8)
P1 = 1                    # GEMM1 k-slab PAIRS in fp8 DR (of KB1//2 = 32)
SW0 = 1024.0              # power-of-2 prescale on W0 (both dtypes)
SW1 = 2048.0              # power-of-2 prescale on W1 (both dtypes)
F16_1 = KB1 - 2 * P1      # fp16 k-slabs in GEMM1

E4 = ml_dtypes.float8_e4m3

TRACE = False             # set True by test harness for NTFF profiling
LAST_EXEC_NS = None       # populated when TRACE

_CACHED = {}


def _build_nc():
    fp16 = mybir.dt.float16
    fp8 = mybir.dt.float8e4
    f32 = mybir.dt.float32
    gelu = mybir.ActivationFunctionType.Gelu
    ident = mybir.ActivationFunctionType.Identity
    DR = mybir.MatmulPerfMode.DoubleRow

    nc = bacc.Bacc("TRN2", target_bir_lowering=False, debug=False,
                   num_devices=N_CORES)
    x8 = nc.declare_dram_parameter("x8", [P, KB0, M_CORE], fp8,
                                   isOutput=False)
    x16 = nc.declare_dram_parameter("x16", [P, KB0, M_CORE], fp16,
                                    isOutput=False)
    w0q = nc.declare_dram_parameter("w0q", [NBQ, P, KP0, 2, P], fp8,
                                    isOutput=False)
    w0f = nc.declare_dram_parameter("w0f", [NBF, P, KB0, P], fp16,
                                    isOutput=False)
    w1q = nc.declare_dram_parameter("w1q", [HB, P, P1, 2, P], fp8,
                                    isOutput=False)
    w1f = nc.declare_dram_parameter("w1f", [HB, P, F16_1, P], fp16,
                                    isOutput=False)
    b0 = nc.declare_dram_parameter("b0", [P, NB], f32, isOutput=False)
    b1 = nc.declare_dram_parameter("b1", [P, HB], f32, isOutput=False)
    out = nc.declare_dram_parameter("out", [HB, P, M_CORE], fp16,
                                    isOutput=True)

    with tile.TileContext(nc) as tc:
        with (
            tc.tile_pool(name="const", bufs=1) as const_pool,
            tc.tile_pool(name="xp", bufs=1) as x_pool,
            tc.tile_pool(name="dp", bufs=1) as d_pool,
            tc.tile_pool(name="w0p", bufs=4) as w0_pool,
            tc.tile_pool(name="w1p", bufs=3) as w1_pool,
            tc.tile_pool(name="op", bufs=4) as o_pool,
            tc.tile_pool(name="psp", bufs=4, space="PSUM") as ps_pool,
            tc.tile_pool(name="pst", bufs=1, space="PSUM") as pst_pool,
        ):
            x8_sb = x_pool.tile([P, KB0, M_CORE], fp8)
            x16_sb = x_pool.tile([P, KB0, M_CORE], fp16)
            d16_sb = d_pool.tile([P, KB1, M_CORE], fp16)
            d8_sb = d_pool.tile([P, 2 * P1, M_CORE], fp8)
            w0q_first = w0_pool.tile([P, KP0, 2, P], fp8, tag="w0q_sb")

            # Startup: only the fp8 stream (x8 chunks + w0q[0], ~0.6MB)
            # gates the PE; the fp16 x and first fp16 weight block are
            # issued next but aren't needed until block 16 (~27us of DR
            # work later).
            # The first DR matmul needs only w0q[0]'s pair 0 (32KB) +
            # x8's first two pairs (256KB); landing those ahead of the
            # rest pulls the first matmul ~3us earlier, and the
            # remaining chunks stream in at the ramp-phase consumption
            # pace.
            nc.sync.dma_start(out=w0q_first[:, :1, :, :],
                              in_=w0q[0, :, :1, :, :])
            nc.sync.dma_start(out=x8_sb[:, :4, :], in_=x8[:, :4, :])
            nc.sync.dma_start(out=w0q_first[:, 1:, :, :],
                              in_=w0q[0, :, 1:, :, :])
            nc.sync.dma_start(out=x8_sb[:, 4:8, :], in_=x8[:, 4:8, :])
            nc.sync.dma_start(out=x8_sb[:, 8:12, :], in_=x8[:, 8:12, :])
            nc.sync.dma_start(out=x8_sb[:, 12:, :], in_=x8[:, 12:, :])
            b0_sb = const_pool.tile([P, NB], f32)
            nc.sync.dma_start(out=b0_sb[:], in_=b0[:])
            b1_sb = const_pool.tile([P, HB], f32)

            # GEMM0 + bias + gelu -> D^T resident in SBUF
            for nb in range(NB):
                ps = ps_pool.tile([P, M_CORE], f32)
                if nb < NBQ:
                    if nb == 0:
                        w0q_sb = w0q_first
                    else:
                        w0q_sb = w0_pool.tile([P, KP0, 2, P], fp8,
                                              tag="w0q_sb")
                        nc.sync.dma_start(out=w0q_sb[:], in_=w0q[nb])
                        # The 2MB fp16 x (needed only from block NBQ,
                        # ~27us in) rides BEHIND the fp8 weight stream
                        # on the FIFO queue so it never gates it.
                        if nb == 6:
                            nc.sync.dma_start(
                                out=x16_sb[:, :KB0 // 2, :],
                                in_=x16[:, :KB0 // 2, :])
                        elif nb == 11:
                            nc.sync.dma_start(
                                out=x16_sb[:, KB0 // 2:, :],
                                in_=x16[:, KB0 // 2:, :])
                        elif nb == 13:
                            nc.sync.dma_start(out=b1_sb[:], in_=b1[:])
                    for i in range(KP0):
                        nc.tensor.matmul(
                            ps[:],
                            lhsT=w0q_sb[:, i, :, :],
                            rhs=x8_sb[:, 2 * i:2 * i + 2, :],
                            start=(i == 0), stop=(i == KP0 - 1),
                            perf_mode=DR,
                        )
                else:
                    w0f_sb = w0_pool.tile([P, KB0, P], fp16, tag="w0f_sb")
                    nc.sync.dma_start(out=w0f_sb[:], in_=w0f[nb - NBQ])
                    for kb in range(KB0):
                        nc.tensor.matmul(
                            ps[:],
                            lhsT=w0f_sb[:, kb, :],
                            rhs=x16_sb[:, kb, :],
                            start=(kb == 0), stop=(kb == KB0 - 1),
                        )
                nc.scalar.activation(
                    d16_sb[:, nb, :], ps[:], gelu,
                    bias=b0_sb[:, nb:nb + 1], scale=1.0 / SW0,
                )
                if nb < 2 * P1:
                    nc.vector.tensor_copy(d8_sb[:, nb, :], d16_sb[:, nb, :])

            # GEMM1 + bias -> E^T, streamed out. Last block split along
            # tokens so its activation+DMA overlaps the trailing
            # matmuls instead of draining serially at the end.
            for hb in range(HB):
                w1q_sb = w1_pool.tile([P, P1, 2, P], fp8, tag="w1q_sb")
                w1f_sb = w1_pool.tile([P, F16_1, P], fp16, tag="w1f_sb")
                nc.sync.dma_start(out=w1q_sb[:], in_=w1q[hb])
                nc.sync.dma_start(out=w1f_sb[:], in_=w1f[hb])
                halves = ((0, M_CORE),) if hb < HB - 1 else (
                    (0, 256), (256, 448), (448, M_CORE))
                for (m0, m1) in halves:
                    if len(halves) == 1:
                        ps = ps_pool.tile([P, M_CORE], f32)
                    else:
                        ps = pst_pool.tile([P, m1 - m0], f32, tag=f"t{m0}")
                    for i in range(P1):
                        nc.tensor.matmul(
                            ps[:],
                            lhsT=w1q_sb[:, i, :, :],
                            rhs=d8_sb[:, 2 * i:2 * i + 2, m0:m1],
                            start=(i == 0), stop=False,
                            perf_mode=DR,
                        )
                    for kb in range(F16_1):
                        nc.tensor.matmul(
                            ps[:],
                            lhsT=w1f_sb[:, kb, :],
                            rhs=d16_sb[:, 2 * P1 + kb, m0:m1],
                            start=False, stop=(kb == F16_1 - 1),
                        )
                    o_sb = o_pool.tile([P, m1 - m0], fp16)
                    nc.scalar.activation(
                        o_sb[:], ps[:], ident,
                        bias=b1_sb[:, hb:hb + 1], scale=1.0 / SW1,
                    )
                    # The very last chunk's store is issued from the
                    # scalar engine: it follows the activation in
                    # program order, skipping a cross-engine semaphore
                    # hop on the kernel's critical tail.
                    eng = nc.scalar if (hb == HB - 1 and m1 == M_CORE) \
                        else nc.sync
                    eng.dma_start(out=out[hb, :, m0:m1], in_=o_sb[:])

    nc.compile()
    return nc


def kernel(x, W0, bias0, W1, bias1):
    global LAST_EXEC_NS

    if "nc" not in _CACHED:
        _CACHED["nc"] = _build_nc()
    nc = _CACHED["nc"]

    x, W0, bias0, W1, bias1 = (
        np.asarray(t) for t in (x, W0, bias0, W1, bias1))
    X = np.ascontiguousarray(x.reshape(M, H)).astype(np.float16)

    # weights: [out, in] -> [blk, P_contract, slab, P_out]
    w0_all = W0.reshape(NB, P, KB0, P).transpose(0, 3, 2, 1) * SW0
    w0q_host = np.ascontiguousarray(
        w0_all[:NBQ]).reshape(NBQ, P, KP0, 2, P).astype(E4)
    w0f_host = np.ascontiguousarray(w0_all[NBQ:]).astype(np.float16)
    w1_all = W1.reshape(HB, P, KB1, P).transpose(0, 3, 2, 1) * SW1
    w1q_host = np.ascontiguousarray(
        w1_all[:, :, :2 * P1, :]).reshape(HB, P, P1, 2, P).astype(E4)
    w1f_host = np.ascontiguousarray(w1_all[:, :, 2 * P1:, :]).astype(
        np.float16)
    b0_host = np.ascontiguousarray(bias0.astype(np.float32).reshape(NB, P).T)
    b1_host = np.ascontiguousarray(bias1.astype(np.float32).reshape(HB, P).T)

    in_maps = []
    for c in range(N_CORES):
        xs = X[c * M_CORE:(c + 1) * M_CORE]          # (512, 2048)
        xT_host = np.ascontiguousarray(
            xs.T.reshape(KB0, P, M_CORE).transpose(1, 0, 2))
        in_maps.append({
            "x8": xT_host.astype(E4), "x16": xT_host,
            "w0q": w0q_host, "w0f": w0f_host,
            "w1q": w1q_host, "w1f": w1f_host,
            "b0": b0_host, "b1": b1_host,
        })

    res = run_bass_kernel_spmd(
        nc, in_maps, core_ids=list(range(N_CORES)), trace=TRACE)
    if TRACE:
        LAST_EXEC_NS = res.exec_time_ns

    E = np.empty((M, H), dtype=np.float16)
    for c in range(N_CORES):
        o = res.results[c]["out"]                    # (HB, P, M_CORE)
        E[c * M_CORE:(c + 1) * M_CORE] = o.transpose(2, 0, 1).reshape(M_CORE, H)
    return E.reshape(B, L, H)


# revision 24
# speedup vs baseline: 1.0153x; 1.0153x over previous
"""Trainium2 Bass kernel for CuteInferMLP: E = gelu(X @ W0^T + b0) @ W1^T + b1.

Full shapes: x (2, 2048, 2048) f32, W0 (8192, 2048), b0 (8192,),
W1 (2048, 8192), b1 (2048,). Output (2, 2048, 2048) f16.

Sharding: 8-way data-parallel over the 4096 tokens (512 tokens/core).
Each core holds the full weights and computes its token slice end to
end; the host just concatenates the 8 slices.

The kernel is PE-bound (fp16 matmul streams 1 moving row/cycle;
fp8e4m3 DoubleRow covers 2 k-slabs per instruction at the same row
rate = 2x MACs). Full fp8 fails the 2e-2 accuracy gate (~5% rel), so
a quarter of GEMM0's MACs and 1/32 of GEMM1's run in fp8 — sized so
total rel err ~1.85% < 2e-2. Quantization-error energy is additive
and position-independent, so GEMM0's fp8 budget is packed into its
FIRST 16 of 64 output blocks (fully fp8) instead of spread across all
blocks: the first ~27us of PE work then depends only on ~1.25MB of
fp8 inputs, taking the 2MB fp16 x stream off the startup critical
path entirely. Weight planes are pre-scaled by powers of two (exact
in fp16 and e4m3) so fp8 and fp16 partial sums share one PSUM scale,
dequantized for free via the activation's scale parameter.

Startup schedule (measured-optimal): the PE clock ramps over its
first ~5us of activity, so early work runs at ~60% speed. Keep early
PE work minimal and let DMA waits overlap the slow-clock phase;
front-running extra work into the ramp window consistently lost time.

Device layout per core (weights stationary, contraction on partitions):
  GEMM0: D^T[n,m] += W0T[h,n]-stationary @ X^T[h,m]   (h: 16 k-slabs)
         blocks n<16: 8 fp8 DR instrs; blocks n>=16: 16 fp16 matmuls
  act:   D^T = gelu(PSUM/1024 + b0) -> fp16 (ScalarE); first 2 blocks
         also cast to fp8 (VectorE) for GEMM1's DR instructions.
  GEMM1: E^T[hh,m] += W1T[n,hh]-stationary @ D^T[n,m] (n: 64 k-slabs,
         first pair fp8 DR, rest fp16)
  act:   E^T = PSUM/2048 + b1 -> fp16, DMA out. Last block split in
         three along tokens so its activation+DMA overlaps matmuls.
"""

import numpy as np
import ml_dtypes

from concourse import bacc, tile, mybir
from concourse.bass_utils import run_bass_kernel_spmd

P = 128
N_CORES = 8
B, L, H, N = 2, 2048, 2048, 8192
M = B * L                 # 4096 tokens
M_CORE = M // N_CORES     # 512 tokens per core
KB0 = H // P              # 16  k-slabs in GEMM0 (contraction over H)
NB = N // P               # 64  n-blocks (GEMM0 output partitions)
KB1 = N // P              # 64  k-slabs in GEMM1 (contraction over N)
HB = H // P               # 16  output blocks (GEMM1 output partitions)

NBQ = 16                  # GEMM0 blocks computed fully in fp8 DR
NBF = NB - NBQ            # GEMM0 blocks computed fully in fp16
KP0 = KB0 // 2            # DR k-pair count per fp8 block (8)
P1 = 1                    # GEMM1 k-slab PAIRS in fp8 DR (of KB1//2 = 32)
SW0 = 1024.0              # power-of-2 prescale on W0 (both dtypes)
SW1 = 2048.0              # power-of-2 prescale on W1 (both dtypes)
F16_1 = KB1 - 2 * P1      # fp16 k-slabs in GEMM1

E4 = ml_dtypes.float8_e4m3

TRACE = False             # set True by test harness for NTFF profiling
LAST_EXEC_NS = None       # populated when TRACE

_CACHED = {}


def _build_nc():
    fp16 = mybir.dt.float16
    fp8 = mybir.dt.float8e4
    f32 = mybir.dt.float32
    gelu = mybir.ActivationFunctionType.Gelu
    ident = mybir.ActivationFunctionType.Identity
    DR = mybir.MatmulPerfMode.DoubleRow

    nc = bacc.Bacc("TRN2", target_bir_lowering=False, debug=False,
                   num_devices=N_CORES)
    x8 = nc.declare_dram_parameter("x8", [P, KB0, M_CORE], fp8,
                                   isOutput=False)
    x16 = nc.declare_dram_parameter("x16", [P, KB0, M_CORE], fp16,
                                    isOutput=False)
    w0q = nc.declare_dram_parameter("w0q", [NBQ, P, KP0, 2, P], fp8,
                                    isOutput=False)
    w0f = nc.declare_dram_parameter("w0f", [NBF, P, KB0, P], fp16,
                                    isOutput=False)
    w1q = nc.declare_dram_parameter("w1q", [HB, P, P1, 2, P], fp8,
                                    isOutput=False)
    w1f = nc.declare_dram_parameter("w1f", [HB, P, F16_1, P], fp16,
                                    isOutput=False)
    b0 = nc.declare_dram_parameter("b0", [P, NB], f32, isOutput=False)
    b1 = nc.declare_dram_parameter("b1", [P, HB], f32, isOutput=False)
    out = nc.declare_dram_parameter("out", [HB, P, M_CORE], fp16,
                                    isOutput=True)

    with tile.TileContext(nc) as tc:
        with (
            tc.tile_pool(name="const", bufs=1) as const_pool,
            tc.tile_pool(name="xp", bufs=1) as x_pool,
            tc.tile_pool(name="dp", bufs=1) as d_pool,
            tc.tile_pool(name="w0p", bufs=4) as w0_pool,
            tc.tile_pool(name="w1p", bufs=3) as w1_pool,
            tc.tile_pool(name="op", bufs=4) as o_pool,
            tc.tile_pool(name="psp", bufs=4, space="PSUM") as ps_pool,
            tc.tile_pool(name="pst", bufs=1, space="PSUM") as pst_pool,
        ):
            x8_sb = x_pool.tile([P, KB0, M_CORE], fp8)
            x16_sb = x_pool.tile([P, KB0, M_CORE], fp16)
            d16_sb = d_pool.tile([P, KB1, M_CORE], fp16)
            d8_sb = d_pool.tile([P, 2 * P1, M_CORE], fp8)
            w0q_first = w0_pool.tile([P, KP0, 2, P], fp8, tag="w0q_sb")

            # Startup: only the fp8 stream (x8 chunks + w0q[0], ~0.6MB)
            # gates the PE; the fp16 x and first fp16 weight block are
            # issued next but aren't needed until block 16 (~27us of DR
            # work later).
            nc.sync.dma_start(out=x8_sb[:, :4, :], in_=x8[:, :4, :])
            nc.sync.dma_start(out=w0q_first[:], in_=w0q[0])
            nc.sync.dma_start(out=x8_sb[:, 4:8, :], in_=x8[:, 4:8, :])
            nc.sync.dma_start(out=x8_sb[:, 8:12, :], in_=x8[:, 8:12, :])
            nc.sync.dma_start(out=x8_sb[:, 12:, :], in_=x8[:, 12:, :])
            b0_sb = const_pool.tile([P, NB], f32)
            nc.sync.dma_start(out=b0_sb[:], in_=b0[:])
            b1_sb = const_pool.tile([P, HB], f32)

            # GEMM0 + bias + gelu -> D^T resident in SBUF
            for nb in range(NB):
                ps = ps_pool.tile([P, M_CORE], f32)
                if nb < NBQ:
                    if nb == 0:
                        w0q_sb = w0q_first
                    else:
                        w0q_sb = w0_pool.tile([P, KP0, 2, P], fp8,
                                              tag="w0q_sb")
                        nc.sync.dma_start(out=w0q_sb[:], in_=w0q[nb])
                        # The 2MB fp16 x (needed only from block NBQ,
                        # ~27us in) rides BEHIND the fp8 weight stream
                        # on the FIFO queue so it never gates it.
                        if nb == 6:
                            nc.sync.dma_start(
                                out=x16_sb[:, :KB0 // 2, :],
                                in_=x16[:, :KB0 // 2, :])
                        elif nb == 11:
                            nc.sync.dma_start(
                                out=x16_sb[:, KB0 // 2:, :],
                                in_=x16[:, KB0 // 2:, :])
                        elif nb == 13:
                            nc.sync.dma_start(out=b1_sb[:], in_=b1[:])
                    for i in range(KP0):
                        nc.tensor.matmul(
                            ps[:],
                            lhsT=w0q_sb[:, i, :, :],
                            rhs=x8_sb[:, 2 * i:2 * i + 2, :],
                            start=(i == 0), stop=(i == KP0 - 1),
                            perf_mode=DR,
                        )
                else:
                    w0f_sb = w0_pool.tile([P, KB0, P], fp16, tag="w0f_sb")
                    nc.sync.dma_start(out=w0f_sb[:], in_=w0f[nb - NBQ])
                    for kb in range(KB0):
                        nc.tensor.matmul(
                            ps[:],
                            lhsT=w0f_sb[:, kb, :],
                            rhs=x16_sb[:, kb, :],
                            start=(kb == 0), stop=(kb == KB0 - 1),
                        )
                nc.scalar.activation(
                    d16_sb[:, nb, :], ps[:], gelu,
                    bias=b0_sb[:, nb:nb + 1], scale=1.0 / SW0,
                )
                if nb < 2 * P1:
                    nc.vector.tensor_copy(d8_sb[:, nb, :], d16_sb[:, nb, :])

            # GEMM1 + bias -> E^T, streamed out. Last block split along
            # tokens so its activation+DMA overlaps the trailing
            # matmuls instead of draining serially at the end.
            for hb in range(HB):
                w1q_sb = w1_pool.tile([P, P1, 2, P], fp8, tag="w1q_sb")
                w1f_sb = w1_pool.tile([P, F16_1, P], fp16, tag="w1f_sb")
                nc.sync.dma_start(out=w1q_sb[:], in_=w1q[hb])
                nc.sync.dma_start(out=w1f_sb[:], in_=w1f[hb])
                halves = ((0, M_CORE),) if hb < HB - 1 else (
                    (0, 256), (256, 384), (384, M_CORE))
                for (m0, m1) in halves:
                    if len(halves) == 1:
                        ps = ps_pool.tile([P, M_CORE], f32)
                    else:
                        ps = pst_pool.tile([P, m1 - m0], f32, tag=f"t{m0}")
                    for i in range(P1):
                        nc.tensor.matmul(
                            ps[:],
                            lhsT=w1q_sb[:, i, :, :],
                            rhs=d8_sb[:, 2 * i:2 * i + 2, m0:m1],
                            start=(i == 0), stop=False,
                            perf_mode=DR,
                        )
                    for kb in range(F16_1):
                        nc.tensor.matmul(
                            ps[:],
                            lhsT=w1f_sb[:, kb, :],
                            rhs=d16_sb[:, 2 * P1 + kb, m0:m1],
                            start=False, stop=(kb == F16_1 - 1),
                        )
                    o_sb = o_pool.tile([P, m1 - m0], fp16)
                    nc.scalar.activation(
                        o_sb[:], ps[:], ident,
                        bias=b1_sb[:, hb:hb + 1], scale=1.0 / SW1,
                    )
                    # The very last chunk's store is issued from the
                    # scalar engine: it follows the activation in
                    # program order, skipping a cross-engine semaphore
                    # hop on the kernel's critical tail.
                    eng = nc.scalar if (hb == HB - 1 and m1 == M_CORE) \
                        else nc.sync
                    eng.dma_start(out=out[hb, :, m0:m1], in_=o_sb[:])

    nc.compile()
    return nc


def kernel(x, W0, bias0, W1, bias1):
    global LAST_EXEC_NS

    if "nc" not in _CACHED:
        _CACHED["nc"] = _build_nc()
    nc = _CACHED["nc"]

    x, W0, bias0, W1, bias1 = (
        np.asarray(t) for t in (x, W0, bias0, W1, bias1))
    X = np.ascontiguousarray(x.reshape(M, H)).astype(np.float16)

    # weights: [out, in] -> [blk, P_contract, slab, P_out]
    w0_all = W0.reshape(NB, P, KB0, P).transpose(0, 3, 2, 1) * SW0
    w0q_host = np.ascontiguousarray(
        w0_all[:NBQ]).reshape(NBQ, P, KP0, 2, P).astype(E4)
    w0f_host = np.ascontiguousarray(w0_all[NBQ:]).astype(np.float16)
    w1_all = W1.reshape(HB, P, KB1, P).transpose(0, 3, 2, 1) * SW1
    w1q_host = np.ascontiguousarray(
        w1_all[:, :, :2 * P1, :]).reshape(HB, P, P1, 2, P).astype(E4)
    w1f_host = np.ascontiguousarray(w1_all[:, :, 2 * P1:, :]).astype(
        np.float16)
    b0_host = np.ascontiguousarray(bias0.astype(np.float32).reshape(NB, P).T)
    b1_host = np.ascontiguousarray(bias1.astype(np.float32).reshape(HB, P).T)

    in_maps = []
    for c in range(N_CORES):
        xs = X[c * M_CORE:(c + 1) * M_CORE]          # (512, 2048)
        xT_host = np.ascontiguousarray(
            xs.T.reshape(KB0, P, M_CORE).transpose(1, 0, 2))
        in_maps.append({
            "x8": xT_host.astype(E4), "x16": xT_host,
            "w0q": w0q_host, "w0f": w0f_host,
            "w1q": w1q_host, "w1f": w1f_host,
            "b0": b0_host, "b1": b1_host,
        })

    res = run_bass_kernel_spmd(
        nc, in_maps, core_ids=list(range(N_CORES)), trace=TRACE)
    if TRACE:
        LAST_EXEC_NS = res.exec_time_ns

    E = np.empty((M, H), dtype=np.float16)
    for c in range(N_CORES):
        o = res.results[c]["out"]                    # (HB, P, M_CORE)
        E[c * M_CORE:(c + 1) * M_CORE] = o.transpose(2, 0, 1).reshape(M_CORE, H)
    return E.reshape(B, L, H)
